# revision 15
# baseline (speedup 1.0000x reference)
"""TRN2 Bass kernel for nn_Attention_11252814315826 (v2: fp16/f32r + dual-engine exp).

out[b,h,s,:] = softmax(Q[b,h] @ K^T[b,h] / 8 + addr(mask)) @ V[b,h]
with the additive mask on the QUERY dim: for mask[b,s]==0 the reference's
-1e12 row offset makes softmax exactly uniform, so out = colmean(V[b,h]).

Device strategy (per core: 4 (b,h) pairs, SP query rows padded to 16):
  - host folds the 1/sqrt(D) scale and the Schraudolph factor into Q:
      m = q' @ k = 1024 * log2(e) * score/8  (bf16 inputs, fp32 PSUM)
  - QK uses 128-partition block-diagonal K weights + row-replicated Q
    (64-partition matmuls pay a ~120ns/instr penalty on TRN2; 2-byte
    weights keep the PE weight double-buffer overlapping)
  - exp via TWO engines in parallel on the fp32 PSUM scores:
      ACT:  exp(m * ln2/1024) -> fp16
      DVE:  int16(max(m + 15*1024 + C, 0)) = fp16 bit pattern of 2^(m/1024)
      (Schraudolph; C calibrated for unbiased weights vs the ACT path;
       int16 saturation at -32768 = fp16 -0.0 handles the low tail)
  - PV in fp16 with a ones-column appended to V for the softmax denominator
  - unnormalized [65, SP] output DMA'd out (input loads on the SP HWDGE
    queue, stores on the ACT queue); host divides + transposes.
Masked (mask==0) query rows are dropped on host; one zero q' row yields
uniform weights -> colmean(V) which host scatters to all masked rows.
"""

import os
import sys

for _p in (
    "/root/.axon_site",
    "/root/.axon_site/_ro/trn_rl_repo",
    "/root/.axon_site/_ro/pypackages",
    "/opt/trn_rl_repo",
):
    if os.path.isdir(_p) and _p not in sys.path:
        sys.path.append(_p)

from concourse.bass_utils import run_bass_kernel_spmd

import numpy as np
import ml_dtypes

import concourse.bacc as bacc
import concourse.tile as tile
import concourse.mybir as mybir

F32 = mybir.dt.float32
F32R = mybir.dt.float32r
BF16 = mybir.dt.bfloat16
FP16 = mybir.dt.float16
I16 = mybir.dt.int16

LOG2E = 1.4426950408889634
ALPHA = 0.125 * LOG2E * 1024.0      # q' = q * ALPHA  ->  m = 1024*u
S_ACT = 1.0 / (1024.0 * LOG2E)      # exp(m * S_ACT) = exp(score/8)
C16 = -59.0
B16 = 1024.0 * 15 + C16             # fp16 Schraudolph bias


def _chunks(SP):
    out, s0 = [], 0
    while s0 < SP:
        w = min(512, SP - s0)
        out.append((s0, w))
        s0 += w
    return out


def build_attention_nc(NP=4, SP=1056, S=2048, D=64, repeat=1):
    assert SP % 16 == 0 and S % 256 == 0 and D == 64
    NT = S // 128

    nc = bacc.Bacc("TRN2", target_bir_lowering=False, debug=False)

    qt = nc.dram_tensor("qt", [NP, 128, SP], BF16, kind="ExternalInput")
    kt = nc.dram_tensor("kt", [NP, 128, S], BF16, kind="ExternalInput")
    v = nc.dram_tensor("v", [NP, 128, NT, D + 1], FP16, kind="ExternalInput")
    o = nc.dram_tensor("o", [NP, D + 1, SP], F32, kind="ExternalOutput")

    chunks = _chunks(SP)
    ctxs = {}

    with tile.TileContext(nc) as tc:
        with (
            tc.tile_pool(name="kt", bufs=2) as kt_pool,
            tc.tile_pool(name="v", bufs=2) as v_pool,
            tc.tile_pool(name="qt", bufs=2) as qt_pool,
            tc.tile_pool(name="exp", bufs=8) as exp_pool,
            tc.tile_pool(name="osb", bufs=3) as osb_pool,
            tc.tile_pool(name="qkps", bufs=3, space="PSUM") as qk_psum,
            tc.tile_pool(name="pvps", bufs=2, space="PSUM") as pv_psum,
        ):
            dma = nc.sync

            def pair_prologue(p):
                kt_sb = kt_pool.tile([128, S], BF16)
                for c0 in range(0, S, S // 4):
                    dma.dma_start(
                        kt_sb[:, c0 : c0 + S // 4], kt.ap()[p][:, c0 : c0 + S // 4]
                    )
                v_sb = v_pool.tile([128, NT, D + 1], FP16)
                for t0 in range(0, NT, NT // 2):
                    dma.dma_start(
                        v_sb[:, t0 : t0 + NT // 2, :], v.ap()[p][:, t0 : t0 + NT // 2, :]
                    )
                qt_sb = qt_pool.tile([128, SP], BF16)
                dma.dma_start(qt_sb[:], qt.ap()[p])
                ctxs[p] = dict(kt=kt_sb, v=v_sb, qt=qt_sb)

            # greedy two-engine load balancer for exp tiles + epilogue copies
            eng_load = {"act": 0.0, "dve": 0.0}

            def emit_qk(p, s0, sw, g):
                cx = ctxs[p]
                qk_ps = qk_psum.tile([128, 2, 512], F32, tag="qkp")
                for half in range(2):
                    t = 2 * g + half
                    nc.tensor.matmul(
                        qk_ps[:, half, 0:sw],
                        cx["kt"][:, t * 128 : (t + 1) * 128],
                        cx["qt"][:, s0 : s0 + sw],
                        start=True,
                        stop=True,
                    )
                return qk_ps

            def emit_exp(p, sw, qk_ps):
                exp_sb = exp_pool.tile([128, 2, 512], FP16, tag="exp")
                cost_act = 2 * sw * 1.314 + 29.0
                cost_dve = 2 * sw * 1.47 + 5.0
                if eng_load["act"] + cost_act <= eng_load["dve"] + cost_dve:
                    eng_load["act"] += cost_act
                    nc.scalar.activation(
                        exp_sb[:, :, 0:sw],
                        qk_ps[:, :, 0:sw],
                        mybir.ActivationFunctionType.Exp,
                        scale=S_ACT,
                    )
                else:
                    eng_load["dve"] += cost_dve
                    nc.vector.tensor_scalar(
                        exp_sb[:, :, 0:sw].bitcast(I16),
                        qk_ps[:, :, 0:sw],
                        B16,
                        0.0,
                        mybir.AluOpType.add,
                        mybir.AluOpType.max,
                    )
                return exp_sb

            def make_pv(p, sw, g, exp_sb, pv_ps):
                def emit():
                    v_sb = ctxs[p]["v"]
                    for half in range(2):
                        t = 2 * g + half
                        nc.tensor.matmul(
                            pv_ps[:, 0:sw],
                            v_sb[:, t, :],
                            exp_sb[:, half, 0:sw],
                            start=(t == 0),
                            stop=(t == NT - 1),
                            skip_group_check=True,
                        )

                return emit

            def make_epilogue(p, s0, sw, pv_ps):
                def emit():
                    o_sb = osb_pool.tile([D + 1, 512], F32, tag="osb")
                    cost_act = sw * 1.314 + 29.0
                    cost_dve = sw * 1.32 + 5.0
                    if eng_load["act"] + cost_act <= eng_load["dve"] + cost_dve:
                        eng_load["act"] += cost_act
                        nc.scalar.activation(
                            o_sb[:, 0:sw],
                            pv_ps[:, 0:sw],
                            mybir.ActivationFunctionType.Copy,
                        )
                    else:
                        eng_load["dve"] += cost_dve
                        nc.vector.tensor_copy(o_sb[:, 0:sw], pv_ps[:, 0:sw])
                    nc.scalar.dma_start(o.ap()[p][:, s0 : s0 + sw], o_sb[:, 0:sw])

                return emit

            def emit_body():
                step = [0]
                pvq = []
                delayed = []

                def tick():
                    step[0] += 1
                    if len(pvq) >= 2:
                        pvq.pop(0)()
                    for due, fn in [d for d in delayed if d[0] <= step[0]]:
                        delayed.remove((due, fn))
                        fn()

                for p in range(NP):
                    pair_prologue(p)
                    for s0, sw in _chunks(SP):
                        pv_ps = pv_psum.tile([D + 1, 512], F32, tag="pvp")
                        for g in range(NT // 2):
                            qk_ps = emit_qk(p, s0, sw, g)
                            exp_sb = emit_exp(p, sw, qk_ps)
                            tick()
                            pvq.append(make_pv(p, sw, g, exp_sb, pv_ps))
                        delayed.append((step[0] + 3, make_epilogue(p, s0, sw, pv_ps)))
                while pvq:
                    pvq.pop(0)()
                for _, fn in delayed:
                    fn()

            if repeat == 1:
                emit_body()
            else:
                with tc.For_i(0, repeat, 1):
                    emit_body()

    nc.compile()
    return nc


B, H = 2, 16
S, D = 2048, 64
N_CORES = 8
PAIRS_PER_CORE = (B * H) // N_CORES  # 4

_NC_CACHE = {}
last_results = None


def _install_profile_hook():
    import types

    try:
        import antenv.axon_hooks  # noqa: F401

        return
    except ImportError:
        pass
    try:
        from trn_agent_boot.trn_boot import _ntff_profile_via_ctypes

        hook = _ntff_profile_via_ctypes("/opt/axon/libaxon_pjrt.so")
    except Exception:
        hook = None
    mod = types.ModuleType("antenv.axon_hooks")
    mod._hook = hook
    mod.get_axon_ntff_profile_hook = lambda: mod._hook
    mod.set_axon_ntff_profile_hook = lambda h: setattr(mod, "_hook", h)
    sys.modules["antenv.axon_hooks"] = mod
    import antenv

    antenv.axon_hooks = mod
    import concourse.bass_utils as _bu

    _bu.upload_artifacts = lambda tmpdir: "local://" + tmpdir


def _prep_inputs(query, key, value, mask):
    """Host-side shard + quantize. Returns (in_maps, idx, cnt, SP)."""
    idx = [np.nonzero(mask[b] != 0)[0] for b in range(B)]
    cnt = [len(ix) for ix in idx]
    need = max(cnt) + (1 if max(cnt) < S else 0)
    SP = max(16, -(-need // 16) * 16)

    qs = (query.astype(np.float32) * np.float32(ALPHA))
    vf = value.astype(np.float16)

    in_maps = []
    NT = S // 128
    for c in range(N_CORES):
        qts = np.zeros((PAIRS_PER_CORE, 128, SP), dtype=ml_dtypes.bfloat16)
        kts = np.zeros((PAIRS_PER_CORE, 128, S), dtype=ml_dtypes.bfloat16)
        vs = np.zeros((PAIRS_PER_CORE, 128, NT, D + 1), dtype=np.float16)
        lo = (np.arange(S) % 128) < 64
        for i in range(PAIRS_PER_CORE):
            pair = c * PAIRS_PER_CORE + i
            b, h = pair // H, pair % H
            qT = qs[b, h][idx[b]].T.astype(ml_dtypes.bfloat16)
            qts[i, 0:D, : cnt[b]] = qT
            qts[i, D:128, : cnt[b]] = qT
            kts[i, 0:D][:, lo] = key[b, h][:, lo].astype(ml_dtypes.bfloat16)
            kts[i, D:128][:, ~lo] = key[b, h][:, ~lo].astype(ml_dtypes.bfloat16)
            vs[i, :, :, 0:D] = vf[b, h].reshape(NT, 128, D).transpose(1, 0, 2)
            vs[i, :, :, D] = np.float16(1.0)
        in_maps.append({"qt": qts, "kt": kts, "v": vs})
    return in_maps, idx, cnt, SP


def kernel(query, key, value, mask):
    """Full-input attention; shards over 8 NeuronCores internally."""
    global last_results
    query = np.ascontiguousarray(np.asarray(query, dtype=np.float32))
    key = np.ascontiguousarray(np.asarray(key, dtype=np.float32))
    value = np.ascontiguousarray(np.asarray(value, dtype=np.float32))
    mask = np.asarray(mask)

    in_maps, idx, cnt, SP = _prep_inputs(query, key, value, mask)

    nc = _NC_CACHE.get(SP)
    if nc is None:
        nc = _NC_CACHE[SP] = build_attention_nc(NP=PAIRS_PER_CORE, SP=SP)

    trace = os.environ.get("KERNEL_PROFILE", "") == "1"
    if trace:
        _install_profile_hook()
        try:
            import jax

            jax.device_put(
                np.zeros((4,), np.float32), jax.devices()[0]
            ).block_until_ready()
        except Exception as e:
            print(f"profile warmup failed ({e}); disabling trace", file=sys.stderr)
            trace = False
    res = run_bass_kernel_spmd(nc, in_maps, core_ids=list(range(N_CORES)), trace=trace)
    last_results = res

    out = np.empty((B, H, S, D), dtype=np.float32)
    for c in range(N_CORES):
        oc = res.results[c]["o"]  # [NP, 65, SP] f32, unnormalized
        for i in range(PAIRS_PER_CORE):
            pair = c * PAIRS_PER_CORE + i
            b, h = pair // H, pair % H
            norm = (oc[i, 0:D, :] / oc[i, D : D + 1, :]).T  # [SP, D]
            out[b, h, idx[b]] = norm[: cnt[b]]
            if cnt[b] < S:
                out[b, h, np.nonzero(mask[b] == 0)[0]] = norm[cnt[b]]
    return out
